# revision 86
# baseline (speedup 1.0000x reference)
"""Trainium2 Bass kernel for a dense transformer block (nn_Block_30262339567972).

Full inputs in, full outputs out. Internally sharded across 8 NeuronCores with
zero collectives: core c = 2*b + j owns two 512-token chunks of batch b
(j=0 -> chunks {0,3}, j=1 -> chunks {1,2}; the pairing balances causal
attention work). The host permutes the sequence per core to [cl, ch, rest]
so the core's own tokens sit at columns 0..1023 of the (feature-major)
activations; causal masks are built for the permuted key order, so the
device program is identical across cores (SPMD). Query chunk 0 attends only
key slots {0,2}; chunk 1 attends all four slots with masks on slots {1,3}.

Everything stays in SBUF (no DRAM spills). All matmuls run in bf16 (weights
converted and packed host-side); layernorm statistics, softmax accumulation
and residuals stay fp32 (x2 bf16). Attention scores are in [k, q] layout;
V carries an appended ones-column so the softmax denominator falls out of
the same PSUM accumulation. rstd = exp(-0.5*ln(var+eps)) keeps the whole
kernel on a single ACT table set.

Schedule (one in-order stream per engine, so emission order shapes the
overlap): LN1 all blocks -> Q -> K/V slots {0,2} -> [attention qc0, with
K/V slots {1,3} interleaved to keep the PE dense under the ACT-bound exp
stream] -> proj+LN2 chunk 0 -> [attention qc1 interleaved with fc1+fc2 of
chunk 0] -> proj+LN2+MLP chunk 1. Within attention, scores for k-block i+1
are emitted before AV of block i so the PE never waits on the mask+exp
chain.
"""

from contextlib import ExitStack

import numpy as np
import ml_dtypes

import concourse.bacc as bacc
import concourse.bass as bass
import concourse.tile as tile
from concourse import mybir
from concourse.bass_utils import run_bass_kernel_spmd
F32 = mybir.dt.float32
F32R = mybir.dt.float32r
BF16 = mybir.dt.bfloat16
P = 128
B, T, C = 4, 2048, 1024
H, D = 16, 64
DFF = 4096
TOWN = 1024            # tokens owned per core
EPS = 1e-5
SCALE = D ** -0.5
NEG = -1e30

KT_C = C // P          # 8 contraction tiles over C
FT_C = C // P          # 8 feature tiles over C
TT_FULL = T // P       # 16 token tiles (full seq)
NGROUP = H // 2        # 8 head-pair groups
ND = DFF // P          # 32 dff tiles

# qc0 attends key slots {0, 2} of the permuted order (kt tiles 0-3, 8-11)
QC0_KT = [0, 1, 2, 3, 8, 9, 10, 11]
# qc1 attends all 16 kt tiles; only slots {1, 3} (k2 2,3,6,7) need masks
QC1_MASKED_K2 = {2: 0, 3: 1, 6: 2, 7: 3}

Ident = mybir.ActivationFunctionType.Identity
Ln = mybir.ActivationFunctionType.Ln
Exp = mybir.ActivationFunctionType.Exp
Relu = mybir.ActivationFunctionType.Relu
Square = mybir.ActivationFunctionType.Square
ADD = mybir.AluOpType.add
SUB = mybir.AluOpType.subtract
MULT = mybir.AluOpType.mult


def _alloc(pool, n, shape, dt, tagpfx, namepfx=None, **kw):
    namepfx = namepfx or tagpfx
    return [
        pool.tile(list(shape), dt, tag=f"{tagpfx}{i}", name=f"{namepfx}{i}",
                  **kw)
        for i in range(n)
    ]


def build_nc():
    nc = bacc.Bacc()
    xT = nc.declare_dram_parameter("xT", [C, T], BF16, isOutput=False)
    mask0 = nc.declare_dram_parameter("mask0", [512, 1024], BF16,
                                      isOutput=False)
    mask1 = nc.declare_dram_parameter("mask1", [512, 1024], BF16,
                                      isOutput=False)
    attn_w = nc.declare_dram_parameter("attn_w", [C, 3 * C], BF16,
                                       isOutput=False)
    q_wp = nc.declare_dram_parameter("q_wp", [NGROUP, P, C], BF16,
                                     isOutput=False)
    k_wp = nc.declare_dram_parameter("k_wp", [NGROUP, P, C], BF16,
                                     isOutput=False)
    attn_b = nc.declare_dram_parameter("attn_b", [3 * C], F32, isOutput=False)
    proj_wp = nc.declare_dram_parameter("proj_wp", [FT_C, P, C], BF16,
                                        isOutput=False)
    proj_b = nc.declare_dram_parameter("proj_b", [C], F32, isOutput=False)
    ln1_g = nc.declare_dram_parameter("ln1_g", [C], F32, isOutput=False)
    ln1_b = nc.declare_dram_parameter("ln1_b", [C], F32, isOutput=False)
    ln2_g = nc.declare_dram_parameter("ln2_g", [C], F32, isOutput=False)
    ln2_b = nc.declare_dram_parameter("ln2_b", [C], F32, isOutput=False)
    fc1_wp = nc.declare_dram_parameter("fc1_wp", [16, P, 2048], BF16,
                                       isOutput=False)
    fc1_b = nc.declare_dram_parameter("fc1_b", [DFF], F32, isOutput=False)
    fc2_wp = nc.declare_dram_parameter("fc2_wp", [FT_C, P, DFF], BF16,
                                       isOutput=False)
    fc2_b = nc.declare_dram_parameter("fc2_b", [C], F32, isOutput=False)
    out = nc.declare_dram_parameter("out", [C, TOWN], F32, isOutput=True)

    with tile.TileContext(nc, pool_alloc_mode="queue") as tc, \
            ExitStack() as top:
        const = top.enter_context(tc.tile_pool(name="const", bufs=1))
        eps_t = const.tile([P, 1], F32, name="eps_t")
        nc.vector.memset(eps_t, EPS)
        ones1b = const.tile([P, 1], BF16, name="ones1b")
        nc.vector.memset(ones1b, 1.0)
        zero_t = const.tile([P, 1], F32, name="zero_t")
        nc.vector.memset(zero_t, 0.0)
        ln1g_t = const.tile([P, FT_C], F32, name="ln1g_t")
        ln1b_t = const.tile([P, FT_C], F32, name="ln1b_t")
        ln2g_t = const.tile([P, FT_C], F32, name="ln2g_t")
        ln2b_t = const.tile([P, FT_C], F32, name="ln2b_t")
        nc.sync.dma_start(out=ln1g_t, in_=ln1_g.rearrange("(f p) -> p f", p=P))
        nc.sync.dma_start(out=ln1b_t, in_=ln1_b.rearrange("(f p) -> p f", p=P))
        nc.sync.dma_start(out=ln2g_t, in_=ln2_g.rearrange("(f p) -> p f", p=P))
        nc.sync.dma_start(out=ln2b_t, in_=ln2_b.rearrange("(f p) -> p f", p=P))
        abq_t = const.tile([P, NGROUP], F32, name="abq_t")
        abk_t = const.tile([P, NGROUP], F32, name="abk_t")
        nc.sync.dma_start(out=abq_t,
                          in_=attn_b[0:C].rearrange("(g p) -> p g", p=P))
        nc.sync.dma_start(out=abk_t,
                          in_=attn_b[C:2 * C].rearrange("(g p) -> p g", p=P))
        projb_t = const.tile([P, FT_C], F32, name="projb_t")
        nc.sync.dma_start(out=projb_t, in_=proj_b.rearrange("(f p) -> p f", p=P))
        fc2b_t = const.tile([P, FT_C], F32, name="fc2b_t")
        nc.sync.dma_start(out=fc2b_t, in_=fc2_b.rearrange("(f p) -> p f", p=P))
        fc1b_t = const.tile([P, ND], F32, name="fc1b_t")
        nc.sync.dma_start(out=fc1b_t, in_=fc1_b.rearrange("(f p) -> p f", p=P))
        bv_bc = const.tile([P, C], F32, name="bv_bc")
        abv = attn_b[2 * C:3 * C]
        nc.sync.dma_start(
            out=bv_bc,
            in_=bass.AP(tensor=abv.tensor, offset=abv.offset,
                        ap=[[0, P]] + list(abv.ap[-1:])))

        # Persistent activation tensors
        kvq = top.enter_context(tc.tile_pool(name="kvq", bufs=1))
        kT = [_alloc(kvq, NGROUP, [P, TOWN], BF16, f"kT{h}_")
              for h in range(2)]
        vall = kvq.tile([P, TT_FULL, NGROUP, 130], BF16, name="vall")
        nc.vector.memset(vall, 1.0)   # ones columns for softmax denominators
        qp1 = top.enter_context(tc.tile_pool(name="qp1", bufs=1))
        atp0 = top.enter_context(tc.tile_pool(name="atp0", bufs=1))
        atp1 = top.enter_context(tc.tile_pool(name="atp1", bufs=1))
        attnT = [_alloc(atp0, NGROUP, [P, 512], BF16, "attnT0_"),
                 _alloc(atp1, NGROUP, [P, 512], BF16, "attnT1_")]

        def ln_block(ctx_pools, x_ap_of, dst, dst_sl, g_col, b_col, pfx,
                     st_tags=("ssum", "ssq"), st_bufs=2):
            """LayerNorm one 512-token block (feature-major, bf16 inputs).

            x_ap_of(kt) -> [P,512] bf16 AP. dst: FT_C tiles, written at
            [:, dst_sl] in bf16. Stats via ones-matmul partition
            reductions; rstd = exp(-0.5*ln(var+eps))."""
            st_ps, rowp, bcp = ctx_pools
            xs = [x_ap_of(kt) for kt in range(KT_C)]
            ssum = st_ps.tile([1, 512], F32, tag=st_tags[0], name=f"{pfx}ss",
                              bufs=st_bufs)
            ssq = st_ps.tile([1, 512], F32, tag=st_tags[1], name=f"{pfx}sq",
                             bufs=st_bufs)
            for kt in range(KT_C):
                nc.tensor.matmul(ssum, ones1b, xs[kt],
                                 start=(kt == 0), stop=(kt == KT_C - 1))
            for kt in range(KT_C):
                sq = rowp.tile([P, 512], BF16, tag="sqt", name=f"{pfx}sqt{kt}",
                               bufs=2)
                nc.vector.tensor_mul(out=sq, in0=xs[kt], in1=xs[kt])
                nc.tensor.matmul(ssq, ones1b, sq,
                                 start=(kt == 0), stop=(kt == KT_C - 1))
            mu = rowp.tile([1, 512], F32, tag="mu", name=f"{pfx}mu", bufs=1)
            nc.vector.tensor_scalar_mul(out=mu, in0=ssum, scalar1=1.0 / C)
            var = rowp.tile([1, 512], F32, tag="var", name=f"{pfx}var",
                            bufs=1)
            nc.vector.tensor_mul(out=var, in0=mu, in1=mu)
            nc.vector.scalar_tensor_tensor(out=var, in0=ssq, scalar=1.0 / C,
                                           in1=var, op0=MULT, op1=SUB)
            nc.scalar.activation(out=var, in_=var, func=Ln,
                                 bias=eps_t[0:1, 0:1], scale=1.0)
            rs = rowp.tile([1, 512], BF16, tag="rs", name=f"{pfx}rs", bufs=1)
            nc.scalar.activation(out=rs, in_=var, func=Exp, scale=-0.5)
            ms = rowp.tile([1, 512], BF16, tag="ms", name=f"{pfx}ms", bufs=1)
            nc.vector.tensor_mul(out=ms, in0=mu, in1=rs)
            rs_b = bcp.tile([P, 512], BF16, tag="rsb", name=f"{pfx}rsb")
            nc.gpsimd.partition_broadcast(rs_b, rs)
            ms_b = bcp.tile([P, 512], BF16, tag="msb", name=f"{pfx}msb")
            nc.gpsimd.partition_broadcast(ms_b, ms)
            for ft in range(FT_C):
                t = rowp.tile([P, 512], BF16, tag="ap", name=f"{pfx}ap{ft}")
                nc.vector.tensor_mul(out=t, in0=xs[ft], in1=rs_b)
                nc.vector.tensor_sub(out=t, in0=t, in1=ms_b)
                nc.scalar.activation(out=dst[ft][:, dst_sl], in_=t,
                                     func=Ident, bias=b_col[:, ft:ft + 1],
                                     scale=g_col[:, ft:ft + 1])

        # ================= Phase A: LN1 (all 4 blocks) =================
        sA = ExitStack()
        hfp = sA.enter_context(tc.tile_pool(name="hfp", bufs=1))
        hT = [_alloc(hfp, FT_C, [P, TOWN], BF16, f"hT{h}_") for h in range(2)]
        with ExitStack() as cl:
            st_ps = cl.enter_context(tc.tile_pool(name="st1", bufs=2,
                                                  space="PSUM"))
            rowp = cl.enter_context(tc.tile_pool(name="rowp1", bufs=3))
            bcp = cl.enter_context(tc.tile_pool(name="bcp1", bufs=2))
            lnp = cl.enter_context(tc.tile_pool(name="lnp", bufs=1))
            for half in range(2):
                for blk in range(2):
                    sl = slice(blk * 512, (blk + 1) * 512)

                    def xload(kt, _h=half, _b=blk):
                        t = lnp.tile([P, 512], BF16, tag=f"xf{kt}",
                                     name=f"xf{_h}_{kt}_{_b}", bufs=2)
                        nc.sync.dma_start(
                            out=t,
                            in_=xT[kt * P:(kt + 1) * P,
                                   _h * TOWN + _b * 512:
                                   _h * TOWN + (_b + 1) * 512])
                        return t[:, :]
                    ln_block((st_ps, rowp, bcp), xload, hT[half], sl,
                             ln1g_t, ln1b_t, f"l1{half}{blk}")

        # ============ Q + K/V projections (emission units) ============
        sW = ExitStack()
        ws = sW.enter_context(tc.tile_pool(name="ws", bufs=1))
        mm_kvq = sW.enter_context(tc.tile_pool(name="mm_kvq", bufs=2,
                                               space="PSUM"))
        # V weights: shared by all slots, resident
        wv = [[None, None] for _ in range(KT_C)]
        for kt in range(KT_C):
            for nb in range(2):
                w = ws.tile([P, 512], BF16, tag=f"wv{kt}_{nb}",
                            name=f"wv{kt}_{nb}", bufs=1)
                nc.sync.dma_start(
                    out=w, in_=attn_w[kt * P:(kt + 1) * P,
                                      2 * C + nb * 512:2 * C + (nb + 1) * 512])
                wv[kt][nb] = w

        sQ0 = ExitStack()
        qp0 = sQ0.enter_context(tc.tile_pool(name="qp0", bufs=1))
        qT = [_alloc(qp0, NGROUP, [P, 512], BF16, "qT0_"),
              _alloc(qp1, NGROUP, [P, 512], BF16, "qT1_")]

        def emit_q():
            wqs = {}

            def qload(g):
                wq = ws.tile([P, C], BF16, tag="wqg", name=f"wq{g}", bufs=2)
                nc.sync.dma_start(out=wq, in_=q_wp[g])
                wqs[g] = wq
            qload(0)
            for g in range(NGROUP):
                wq = wqs.pop(g)
                if g + 1 < NGROUP:
                    qload(g + 1)
                for qc in range(2):
                    ps = mm_kvq.tile([P, 512], F32, tag="mm",
                                     name=f"qps{g}_{qc}")
                    for kt in range(KT_C):
                        nc.tensor.matmul(
                            ps, wq[:, kt * P:(kt + 1) * P],
                            hT[0][kt][:, qc * 512:(qc + 1) * 512],
                            start=(kt == 0), stop=(kt == KT_C - 1))
                    nc.vector.tensor_scalar_add(
                        out=qT[qc][g], in0=ps, scalar1=abq_t[:, g:g + 1])

        def k_units(slots):
            """One unit per g: K columns for the given slots (wk loaded
            once, next group's load pre-issued)."""
            units = []
            wks = {}

            def kload(g, s0):
                wk = ws.tile([P, C], BF16, tag="wkg",
                             name=f"wk{g}_{s0}", bufs=2)
                nc.sync.dma_start(out=wk, in_=k_wp[g])
                wks[g] = wk
            for g in range(NGROUP):
                def go(_g=g, _slots=slots):
                    if _g not in wks:
                        kload(_g, _slots[0])
                    wk = wks.pop(_g)
                    if _g + 1 < NGROUP:
                        kload(_g + 1, _slots[0])
                    for s in _slots:
                        half, nb = s // 2, s % 2
                        ps = mm_kvq.tile([P, 512], F32, tag="mm",
                                         name=f"kps{_g}_{s}")
                        for kt in range(KT_C):
                            nc.tensor.matmul(
                                ps, wk[:, kt * P:(kt + 1) * P],
                                hT[half][kt][:, nb * 512:(nb + 1) * 512],
                                start=(kt == 0), stop=(kt == KT_C - 1))
                        nc.vector.tensor_scalar_add(
                            out=kT[half][_g][:, nb * 512:(nb + 1) * 512],
                            in0=ps, scalar1=abk_t[:, _g:_g + 1])
                units.append(go)
            return units

        def v_units(s):
            """One unit per (tt, nb): V rows for slot s."""
            units = []
            half, snb = s // 2, s % 2
            for tt in range(4):
                ht = snb * 4 + tt            # token tile within the half
                gt = half * FT_C + ht        # global token tile
                for nb in range(2):
                    def go(_ht=ht, _gt=gt, _nb=nb, _half=half):
                        ps = mm_kvq.tile([P, 512], F32, tag="mm",
                                         name=f"vps{_gt}_{_nb}")
                        for kt in range(KT_C):
                            nc.tensor.matmul(
                                ps, hT[_half][kt][:, _ht * P:(_ht + 1) * P],
                                wv[kt][_nb],
                                start=(kt == 0), stop=(kt == KT_C - 1))
                        dst = vall[:, _gt, 4 * _nb:4 * (_nb + 1), :]\
                            .rearrange("p g (h x) -> p (g h) x", h=2)
                        nc.vector.tensor_add(
                            out=dst[:, :, 0:64],
                            in0=ps.rearrange("p (h d) -> p h d", d=64),
                            in1=bv_bc[:, _nb * 512:(_nb + 1) * 512]
                            .rearrange("p (h d) -> p h d", d=64))
                    units.append(go)
            return units

        emit_q()
        for u in k_units([0, 2]):
            u()
        for u in v_units(0):
            u()
        for u in v_units(2):
            u()
        # K/V for slots 1 and 3: interleaved into attention qc0 below
        kv13 = k_units([1, 3]) + v_units(1) + v_units(3)

        # ================= attention machinery =================

        def attn_phase(qc, kts, masked, mtiles, sc_ps, y_pool, ppool,
                       npool, between=None):
            """Attention for all head-pair groups, one query chunk, as a
            single software pipeline over (g, k2) steps: scores/exp run one
            step ahead of AV across group boundaries, so the in-order PE
            stream never waits on the exp chain or the y-bank recycle.

            kts: global kt list (pairs). masked: {k2 -> mask tile idx}.
            between(g): emission hook after group g's last scores step."""
            nk2 = len(kts) // 2
            ys = {}
            pts = {}

            def emit_scores(g, k2):
                if k2 == 0:
                    ys[g] = [y_pool.tile([65, 512], F32, tag=f"y{hh}",
                                         name=f"y{qc}_{g}_{hh}")
                             for hh in range(2)]
                for hh in range(2):
                    hsl = slice(64 * hh, 64 * (hh + 1))
                    sc = sc_ps.tile([P, 1024], F32, tag="sc",
                                    name=f"sc{qc}_{g}_{k2}_{hh}")
                    for j in range(2):
                        kt = kts[2 * k2 + j]
                        nc.tensor.matmul(
                            sc[:, j * 512:(j + 1) * 512],
                            kT[kt // 8][g][hsl, (kt % 8) * P:(kt % 8 + 1) * P],
                            qT[qc][g][hsl, :], start=True, stop=True,
                            tile_position=(64 * hh, 0))
                    pt = ppool.tile([P, 1024], BF16, tag="pt",
                                    name=f"pt{qc}_{g}_{k2}_{hh}")
                    nc.scalar.activation(out=pt, in_=sc, func=Exp,
                                         scale=SCALE)
                    if k2 in masked:
                        # multiplicative {0,1} mask on the exp'd probs
                        # (bf16 SBUF mul; the ones-column denominator sees
                        # masked values, so softmax stays exact)
                        nc.vector.tensor_mul(out=pt, in0=pt,
                                             in1=mtiles[masked[k2]])
                    pts[(g, k2, hh)] = pt

            def emit_av(g, k2):
                for hh in range(2):
                    pt = pts.pop((g, k2, hh))
                    for j in range(2):
                        kt = kts[2 * k2 + j]
                        nc.tensor.matmul(
                            ys[g][hh], vall[:, kt, g, 65 * hh:65 * hh + 65],
                            pt[:, j * 512:(j + 1) * 512],
                            start=(k2 == 0 and j == 0),
                            stop=(k2 == nk2 - 1 and j == 1))
                if k2 == nk2 - 1:
                    y = ys.pop(g)
                    for hh in range(2):
                        # copy out of PSUM first (frees the bank for the
                        # next group's AV). The denominator row goes to a
                        # partition-0 f32 tile: the custom-DVE reciprocal
                        # mis-lowers at base_partition 64 (2.8e6 rel err
                        # on HW), and needs an SBUF f32 operand.
                        yr = npool.tile([1, 512], F32, tag="r",
                                        name=f"yr{qc}_{g}_{hh}", bufs=2)
                        nc.vector.tensor_copy(out=yr, in_=y[hh][64:65, :])
                        yc = npool.tile([64, 512], BF16, tag="yc",
                                        name=f"yc{qc}_{g}_{hh}", bufs=2)
                        nc.vector.tensor_copy(out=yc, in_=y[hh][0:64, :])
                        r = npool.tile([1, 512], F32, tag="r",
                                       name=f"r{qc}_{g}_{hh}", bufs=2)
                        nc.vector.reciprocal_approx_fast(out=r, in_=yr)
                        rb = npool.tile([64, 512], F32, tag="rb",
                                        name=f"rb{qc}_{g}_{hh}")
                        nc.gpsimd.partition_broadcast(rb, r[0:1, :])
                        nc.vector.tensor_mul(
                            out=attnT[qc][g][64 * hh:64 * (hh + 1), :],
                            in0=yc, in1=rb)

            steps = [(g, k2) for g in range(NGROUP) for k2 in range(nk2)]
            prev = None
            for g, k2 in steps:
                emit_scores(g, k2)
                if prev is not None:
                    emit_av(*prev)
                prev = (g, k2)
                if between is not None and k2 == nk2 - 1:
                    between(g)
            emit_av(*prev)

        # ============== MLP chain (emission units, lazy tiles) ==============
        x2T = [None, None]
        h2T = [None, None]
        h1T = [None] * ND

        def mlp_units(qc, pools):
            """proj + LN2 + fc1 + fc2 for one 512-token chunk as emission
            units. Weight/input DMAs for unit i+1 are issued before unit
            i's compute so the DMA queues run a unit ahead of the PE."""
            (xop, mlp_ps, w1p, w2p, h1p, x2p, h2p, op, pwp) = pools
            units = []
            ld = {}

            def proj_load(ft):
                xo = xop.tile([P, 512], BF16, tag="xo",
                              name=f"xo{qc}_{ft}", bufs=2)
                nc.sync.dma_start(
                    out=xo, in_=xT[ft * P:(ft + 1) * P,
                                   qc * 512:(qc + 1) * 512])
                pwt = pwp.tile([P, C], BF16, tag="pw",
                               name=f"pw{qc}_{ft}", bufs=2)
                nc.sync.dma_start(out=pwt, in_=proj_wp[ft])
                ld[("p", ft)] = (xo, pwt)

            def proj_unit(ft):
                def go():
                    if x2T[qc] is None:
                        x2T[qc] = [None] * FT_C
                    if ("p", ft) not in ld:
                        proj_load(ft)
                    xo, pwt = ld.pop(("p", ft))
                    if ft + 1 < FT_C:
                        proj_load(ft + 1)
                    ps = mlp_ps.tile([P, 512], F32, tag="mm",
                                     name=f"prj{qc}_{ft}")
                    for kt in range(KT_C):
                        nc.tensor.matmul(
                            ps, pwt[:, kt * P:(kt + 1) * P],
                            attnT[qc][kt], start=(kt == 0),
                            stop=(kt == KT_C - 1))
                    x2 = x2p.tile([P, 512], BF16, tag=f"x2_{ft}",
                                  name=f"x2_{qc}_{ft}")
                    x2T[qc][ft] = x2
                    nc.vector.scalar_tensor_tensor(
                        out=x2, in0=ps, scalar=projb_t[:, ft:ft + 1],
                        in1=xo, op0=ADD, op1=ADD)
                return go
            for ft in range(FT_C):
                units.append(proj_unit(ft))

            def ln2_unit():
                h2T[qc] = _alloc(h2p, FT_C, [P, 512], BF16, "h2_",
                                 namepfx=f"h2_{qc}_")
                with ExitStack() as c2:
                    rowp = c2.enter_context(
                        tc.tile_pool(name=f"rowl2{qc}", bufs=2))
                    bcp = c2.enter_context(
                        tc.tile_pool(name=f"bcl2{qc}", bufs=2))
                    ln_block((mlp_ps, rowp, bcp),
                             lambda kt: x2T[qc][kt][:, :], h2T[qc],
                             slice(0, 512), ln2g_t, ln2b_t, f"l2{qc}",
                             st_tags=("mm", "mm"), st_bufs=2)
            units.append(ln2_unit)

            def fc1_load(q16):
                w1 = w1p.tile([P, 2048], BF16, tag="w1",
                              name=f"w1{qc}_{q16}", bufs=2)
                nc.sync.dma_start(out=w1, in_=fc1_wp[q16])
                ld[("1", q16)] = w1

            def fc1_unit(q16):
                def go():
                    if ("1", q16) not in ld:
                        fc1_load(q16)
                    w1 = ld.pop(("1", q16))
                    if q16 + 1 < 16:
                        fc1_load(q16 + 1)
                    for dl in range(2):
                        d = q16 * 2 + dl
                        ps = mlp_ps.tile([P, 512], F32, tag="mm",
                                         name=f"f1{qc}_{d}")
                        for kt in range(KT_C):
                            nc.tensor.matmul(
                                ps,
                                w1[:, kt * 256 + dl * P:
                                   kt * 256 + (dl + 1) * P],
                                h2T[qc][kt], start=(kt == 0),
                                stop=(kt == KT_C - 1))
                        h1 = h1p.tile([P, 512], BF16, tag=f"h1_{d}",
                                      name=f"h1_{qc}_{d}")
                        h1T[d] = h1
                        if qc == 0:
                            # B1: ACT is saturated with attention exps;
                            # bias+relu as one DVE tensor_scalar instead
                            nc.vector.tensor_scalar(
                                out=h1, in0=ps,
                                scalar1=fc1b_t[:, d:d + 1], scalar2=0.0,
                                op0=ADD, op1=mybir.AluOpType.max)
                        else:
                            nc.scalar.activation(out=h1, in_=ps, func=Relu,
                                                 bias=fc1b_t[:, d:d + 1],
                                                 scale=1.0)
                return go
            for q16 in range(16):
                units.append(fc1_unit(q16))

            def fc2_load(ft, dh):
                w2 = w2p.tile([P, 2048], BF16, tag="w2",
                              name=f"w2{qc}_{ft}_{dh}", bufs=3)
                nc.sync.dma_start(
                    out=w2, in_=fc2_wp[ft][:, dh * 2048:(dh + 1) * 2048])
                ld[("2", ft, dh)] = w2

            def fc2_unit(ft):
                def go():
                    for dh in range(2):
                        if ("2", ft, dh) not in ld:
                            fc2_load(ft, dh)
                    if ft + 1 < FT_C:
                        fc2_load(ft + 1, 0)
                    ps = mlp_ps.tile([P, 512], F32, tag="mm",
                                     name=f"f2{qc}_{ft}")
                    for dh in range(2):
                        if ("2", ft, dh) not in ld:
                            fc2_load(ft, dh)
                        w2 = ld.pop(("2", ft, dh))
                        for dl in range(16):
                            d = dh * 16 + dl
                            nc.tensor.matmul(ps, w2[:, dl * P:(dl + 1) * P],
                                             h1T[d], start=(d == 0),
                                             stop=(d == ND - 1))
                    o = op.tile([P, 512], F32, tag="o", name=f"o{qc}_{ft}",
                                bufs=1)
                    nc.vector.scalar_tensor_tensor(
                        out=o, in0=ps, scalar=fc2b_t[:, ft:ft + 1],
                        in1=x2T[qc][ft], op0=ADD, op1=ADD)
                    nc.sync.dma_start(
                        out=out[ft * P:(ft + 1) * P,
                                qc * 512:(qc + 1) * 512], in_=o)
                return go
            for ft in range(FT_C):
                units.append(fc2_unit(ft))
            return units

        # shared pools for the MLP chains (entered at the B0 tail, after the
        # phase-A pools release, so the emission-ordered allocator can reuse
        # their space; tiles allocated lazily)
        sPS = ExitStack()
        sC = ExitStack()

        # ====== Phase B0: attention qc0 (+ K/V slots 1,3) + proj/LN2 qc0 ====
        if True:
            with ExitStack() as ca0:
                mp0 = ca0.enter_context(tc.tile_pool(name="mp0", bufs=1))
                m0 = _alloc(mp0, 4, [P, 1024], BF16, "m0_")
                for k2 in range(4):
                    nc.sync.dma_start(out=m0[k2],
                                      in_=mask0[k2 * P:(k2 + 1) * P, :])
                sc0 = ca0.enter_context(tc.tile_pool(name="sc0", bufs=2,
                                                     space="PSUM"))
                y0 = ca0.enter_context(tc.tile_pool(name="y0", bufs=1,
                                                    space="PSUM"))
                pp0 = ca0.enter_context(tc.tile_pool(name="pp0", bufs=4))
                np0 = ca0.enter_context(tc.tile_pool(name="np0", bufs=2))
                kst = {"i": 0}

                def kv_between(g):
                    nxt = (g + 1) * len(kv13) // NGROUP
                    while kst["i"] < nxt:
                        kv13[kst["i"]]()
                        kst["i"] += 1
                attn_phase(0, QC0_KT, {0: 0, 1: 1, 2: 2, 3: 3}, m0,
                           sc0, y0, pp0, np0, between=kv_between)
            sQ0.close()
            sW.close()   # free hT / wv / K-Q weight streams, mm_kvq PSUM
            sA.close()
            mlp_ps = sPS.enter_context(tc.tile_pool(name="mlp_ps", bufs=2,
                                                    space="PSUM"))
            xop = sC.enter_context(tc.tile_pool(name="xop", bufs=1))
            w1p = sC.enter_context(tc.tile_pool(name="w1p", bufs=1))
            w2p = sC.enter_context(tc.tile_pool(name="w2p", bufs=1))
            h1p = sC.enter_context(tc.tile_pool(name="h1p", bufs=1))
            x2p = sC.enter_context(tc.tile_pool(name="x2p", bufs=1))
            h2p = sC.enter_context(tc.tile_pool(name="h2p", bufs=1))
            op = sC.enter_context(tc.tile_pool(name="op", bufs=1))
            pwp = sC.enter_context(tc.tile_pool(name="pwp", bufs=1))
            pools = (xop, mlp_ps, w1p, w2p, h1p, x2p, h2p, op, pwp)
            units0 = mlp_units(0, pools)

        # == Phase B1: attention qc1 interleaved with the whole qc0 chain ==
        if True:
            with ExitStack() as ca1:
                mp1 = ca1.enter_context(tc.tile_pool(name="mp1", bufs=1))
                m1 = _alloc(mp1, 4, [P, 1024], BF16, "m1_")
                for i in range(4):
                    nc.sync.dma_start(out=m1[i],
                                      in_=mask1[i * P:(i + 1) * P, :])
                sc1 = ca1.enter_context(tc.tile_pool(name="sc1", bufs=2,
                                                     space="PSUM"))
                y1 = ca1.enter_context(tc.tile_pool(name="y1", bufs=1,
                                                    space="PSUM"))
                pp1 = ca1.enter_context(tc.tile_pool(name="pp1", bufs=3))
                np1 = ca1.enter_context(tc.tile_pool(name="np1", bufs=2))
                # last 4 fc2(qc0) units held back to fill the C prologue
                rest = units0[:-4]
                held = units0[-4:]
                ust = {"i": 0}

                def mlp_between(g):
                    nxt = (g + 1) * len(rest) // NGROUP
                    while ust["i"] < nxt:
                        rest[ust["i"]]()
                        ust["i"] += 1
                attn_phase(1, list(range(16)), QC1_MASKED_K2, m1,
                           sc1, y1, pp1, np1, between=mlp_between)
                while ust["i"] < len(rest):
                    rest[ust["i"]]()
                    ust["i"] += 1

            # ---- Phase C: proj + LN2 + MLP for chunk 1, with the held
            # back qc0 units filling the proj/LN2 dependency pocket ----
            units1 = mlp_units(1, pools)
            for i, u in enumerate(units1[:9]):
                u()
                if i < len(held):
                    held[i]()
            for u in units1[9:]:
                u()
        sC.close()
        sPS.close()

    nc.compile()
    return nc


_NC_CACHE = None


def _get_nc():
    global _NC_CACHE
    if _NC_CACHE is None:
        _NC_CACHE = build_nc()
    return _NC_CACHE


# core c = 2*b + j; j -> (cl, ch); perm = [cl, ch, rest ascending]
_CHUNKS = {0: (0, 3), 1: (1, 2)}
_PERMS = {0: (0, 3, 1, 2), 1: (1, 2, 0, 3)}


def _pair_mask(m):
    # [1024, 512] -> [512, 1024]: row-block k2 holds [mask(2*k2) | mask(2*k2+1)]
    return np.ascontiguousarray(
        m.reshape(4, 2, 128, 512).transpose(0, 2, 1, 3).reshape(512, 1024))


def _make_masks(perm):
    gpos = np.concatenate([np.arange(c * 512, (c + 1) * 512) for c in perm])
    q0 = gpos[0:512]
    q1 = gpos[512:1024]
    k0 = np.concatenate([gpos[0:512], gpos[1024:1536]])
    k1 = np.concatenate([gpos[512:1024], gpos[1536:2048]])
    m0 = np.where(k0[:, None] <= q0[None, :], 1.0, 0.0).astype(np.float32)
    m1 = np.where(k1[:, None] <= q1[None, :], 1.0, 0.0).astype(np.float32)
    return _pair_mask(m0), _pair_mask(m1)


def _run(inputs, trace=False):
    nc = _get_nc()
    xs = {k: np.ascontiguousarray(np.asarray(v), dtype=np.float32)
          for k, v in inputs.items()}
    bf = lambda a: np.ascontiguousarray(a.astype(ml_dtypes.bfloat16))
    aw = xs["attn_w"]
    # [g, p, kt*128+f] = attn_w[kt*128+p, sec + g*128+f]
    q_wp = bf(aw[:, 0:C].reshape(KT_C, P, NGROUP, P)
              .transpose(2, 1, 0, 3).reshape(NGROUP, P, C))
    k_wp = bf(aw[:, C:2 * C].reshape(KT_C, P, NGROUP, P)
              .transpose(2, 1, 0, 3).reshape(NGROUP, P, C))
    # [ft, p, kt*128+f] = proj_w[kt*128+p, ft*128+f]
    proj_wp = bf(xs["proj_w"].reshape(KT_C, P, FT_C, P)
                 .transpose(2, 1, 0, 3).reshape(FT_C, P, C))
    # [q, p, kt*256+f] = fc1_w[kt*128+p, q*256+f]
    fc1_wp = bf(xs["fc1_w"].reshape(KT_C, P, 16, 256)
                .transpose(2, 1, 0, 3).reshape(16, P, 2048))
    # [ft, p, d*128+f] = fc2_w[d*128+p, ft*128+f]
    fc2_wp = bf(xs["fc2_w"].reshape(ND, P, FT_C, P)
                .transpose(2, 1, 0, 3).reshape(FT_C, P, DFF))
    attn_w_bf = bf(aw)
    x = xs["x"]
    in_maps = []
    for c in range(8):
        b, j = divmod(c, 2)
        perm = _PERMS[j]
        m0, m1 = _make_masks(perm)
        xTp = bf(np.concatenate(
            [x[b][p * 512:(p + 1) * 512] for p in perm], axis=0).T)
        in_maps.append({
            "xT": xTp, "mask0": bf(m0), "mask1": bf(m1),
            "attn_w": attn_w_bf, "q_wp": q_wp, "k_wp": k_wp,
            "attn_b": xs["attn_b"],
            "proj_wp": proj_wp, "proj_b": xs["proj_b"],
            "ln1_g": xs["ln1_g"], "ln1_b": xs["ln1_b"],
            "ln2_g": xs["ln2_g"], "ln2_b": xs["ln2_b"],
            "fc1_wp": fc1_wp, "fc1_b": xs["fc1_b"],
            "fc2_wp": fc2_wp, "fc2_b": xs["fc2_b"],
        })
    res = run_bass_kernel_spmd(nc, in_maps, list(range(8)), trace=trace)
    full = np.empty((B, T, C), dtype=np.float32)
    for c in range(8):
        b, j = divmod(c, 2)
        cl, ch = _CHUNKS[j]
        o = res.results[c]["out"]            # [C, TOWN] feature-major
        full[b, cl * 512:(cl + 1) * 512] = o[:, 0:512].T
        full[b, ch * 512:(ch + 1) * 512] = o[:, 512:1024].T
    return full, res.exec_time_ns


def kernel(**inputs):
    out, _ = _run(inputs, trace=False)
    return out


# revision 90
# speedup vs baseline: 1.0104x; 1.0104x over previous
"""Trainium2 Bass kernel for a dense transformer block (nn_Block_30262339567972).

Full inputs in, full outputs out. Internally sharded across 8 NeuronCores with
zero collectives: core c = 2*b + j owns two 512-token chunks of batch b
(j=0 -> chunks {0,3}, j=1 -> chunks {1,2}; the pairing balances causal
attention work). The host permutes the sequence per core to [cl, ch, rest]
so the core's own tokens sit at columns 0..1023 of the (feature-major)
activations; causal masks are built for the permuted key order, so the
device program is identical across cores (SPMD). Query chunk 0 attends only
key slots {0,2}; chunk 1 attends all four slots with masks on slots {1,3}.

Everything stays in SBUF (no DRAM spills). All matmuls run in bf16 (weights
converted and packed host-side); layernorm statistics, softmax accumulation
and residuals stay fp32 (x2 bf16). Attention scores are in [k, q] layout;
V carries an appended ones-column so the softmax denominator falls out of
the same PSUM accumulation. rstd = exp(-0.5*ln(var+eps)) keeps the whole
kernel on a single ACT table set.

Schedule (one in-order stream per engine, so emission order shapes the
overlap): LN1 all blocks -> Q -> K/V slots {0,2} -> [attention qc0, with
K/V slots {1,3} interleaved to keep the PE dense under the ACT-bound exp
stream] -> proj+LN2 chunk 0 -> [attention qc1 interleaved with fc1+fc2 of
chunk 0] -> proj+LN2+MLP chunk 1. Within attention, scores for k-block i+1
are emitted before AV of block i so the PE never waits on the mask+exp
chain.
"""

from contextlib import ExitStack

import numpy as np
import ml_dtypes

import concourse.bacc as bacc
import concourse.bass as bass
import concourse.tile as tile
from concourse import mybir
from concourse.bass_utils import run_bass_kernel_spmd
F32 = mybir.dt.float32
F32R = mybir.dt.float32r
BF16 = mybir.dt.bfloat16
P = 128
B, T, C = 4, 2048, 1024
H, D = 16, 64
DFF = 4096
TOWN = 1024            # tokens owned per core
EPS = 1e-5
SCALE = D ** -0.5
NEG = -1e30

KT_C = C // P          # 8 contraction tiles over C
FT_C = C // P          # 8 feature tiles over C
TT_FULL = T // P       # 16 token tiles (full seq)
NGROUP = H // 2        # 8 head-pair groups
ND = DFF // P          # 32 dff tiles

# qc0 attends key slots {0, 2} of the permuted order (kt tiles 0-3, 8-11)
QC0_KT = [0, 1, 2, 3, 8, 9, 10, 11]
# qc1 attends all 16 kt tiles; only slots {1, 3} (k2 2,3,6,7) need masks
QC1_MASKED_K2 = {2: 0, 3: 1, 6: 2, 7: 3}

Ident = mybir.ActivationFunctionType.Identity
Ln = mybir.ActivationFunctionType.Ln
Exp = mybir.ActivationFunctionType.Exp
Relu = mybir.ActivationFunctionType.Relu
Square = mybir.ActivationFunctionType.Square
ADD = mybir.AluOpType.add
SUB = mybir.AluOpType.subtract
MULT = mybir.AluOpType.mult


def _alloc(pool, n, shape, dt, tagpfx, namepfx=None, **kw):
    namepfx = namepfx or tagpfx
    return [
        pool.tile(list(shape), dt, tag=f"{tagpfx}{i}", name=f"{namepfx}{i}",
                  **kw)
        for i in range(n)
    ]


def build_nc():
    nc = bacc.Bacc()
    xT = nc.declare_dram_parameter("xT", [C, T], BF16, isOutput=False)
    mask0 = nc.declare_dram_parameter("mask0", [512, 1024], BF16,
                                      isOutput=False)
    mask1 = nc.declare_dram_parameter("mask1", [512, 1024], BF16,
                                      isOutput=False)
    attn_w = nc.declare_dram_parameter("attn_w", [C, 3 * C], BF16,
                                       isOutput=False)
    q_wp = nc.declare_dram_parameter("q_wp", [NGROUP, P, C], BF16,
                                     isOutput=False)
    k_wp = nc.declare_dram_parameter("k_wp", [NGROUP, P, C], BF16,
                                     isOutput=False)
    attn_b = nc.declare_dram_parameter("attn_b", [3 * C], F32, isOutput=False)
    proj_wp = nc.declare_dram_parameter("proj_wp", [FT_C, P, C], BF16,
                                        isOutput=False)
    proj_b = nc.declare_dram_parameter("proj_b", [C], F32, isOutput=False)
    ln1_g = nc.declare_dram_parameter("ln1_g", [C], F32, isOutput=False)
    ln1_b = nc.declare_dram_parameter("ln1_b", [C], F32, isOutput=False)
    ln2_g = nc.declare_dram_parameter("ln2_g", [C], F32, isOutput=False)
    ln2_b = nc.declare_dram_parameter("ln2_b", [C], F32, isOutput=False)
    fc1_wp = nc.declare_dram_parameter("fc1_wp", [16, P, 2048], BF16,
                                       isOutput=False)
    fc1_b = nc.declare_dram_parameter("fc1_b", [DFF], F32, isOutput=False)
    fc2_wp = nc.declare_dram_parameter("fc2_wp", [FT_C, P, DFF], BF16,
                                       isOutput=False)
    fc2_b = nc.declare_dram_parameter("fc2_b", [C], F32, isOutput=False)
    out = nc.declare_dram_parameter("out", [C, TOWN], F32, isOutput=True)

    with tile.TileContext(nc, pool_alloc_mode="queue") as tc, \
            ExitStack() as top:
        const = top.enter_context(tc.tile_pool(name="const", bufs=1))
        eps_t = const.tile([P, 1], F32, name="eps_t")
        nc.vector.memset(eps_t, EPS)
        ones1b = const.tile([P, 1], BF16, name="ones1b")
        nc.vector.memset(ones1b, 1.0)
        zero_t = const.tile([P, 1], F32, name="zero_t")
        nc.vector.memset(zero_t, 0.0)
        ln1g_t = const.tile([P, FT_C], F32, name="ln1g_t")
        ln1b_t = const.tile([P, FT_C], F32, name="ln1b_t")
        ln2g_t = const.tile([P, FT_C], F32, name="ln2g_t")
        ln2b_t = const.tile([P, FT_C], F32, name="ln2b_t")
        nc.sync.dma_start(out=ln1g_t, in_=ln1_g.rearrange("(f p) -> p f", p=P))
        nc.sync.dma_start(out=ln1b_t, in_=ln1_b.rearrange("(f p) -> p f", p=P))
        nc.sync.dma_start(out=ln2g_t, in_=ln2_g.rearrange("(f p) -> p f", p=P))
        nc.sync.dma_start(out=ln2b_t, in_=ln2_b.rearrange("(f p) -> p f", p=P))
        abq_t = const.tile([P, NGROUP], F32, name="abq_t")
        abk_t = const.tile([P, NGROUP], F32, name="abk_t")
        nc.sync.dma_start(out=abq_t,
                          in_=attn_b[0:C].rearrange("(g p) -> p g", p=P))
        nc.sync.dma_start(out=abk_t,
                          in_=attn_b[C:2 * C].rearrange("(g p) -> p g", p=P))
        projb_t = const.tile([P, FT_C], F32, name="projb_t")
        nc.sync.dma_start(out=projb_t, in_=proj_b.rearrange("(f p) -> p f", p=P))
        fc2b_t = const.tile([P, FT_C], F32, name="fc2b_t")
        nc.sync.dma_start(out=fc2b_t, in_=fc2_b.rearrange("(f p) -> p f", p=P))
        fc1b_t = const.tile([P, ND], F32, name="fc1b_t")
        nc.sync.dma_start(out=fc1b_t, in_=fc1_b.rearrange("(f p) -> p f", p=P))
        bv_bc = const.tile([P, C], F32, name="bv_bc")
        abv = attn_b[2 * C:3 * C]
        nc.sync.dma_start(
            out=bv_bc,
            in_=bass.AP(tensor=abv.tensor, offset=abv.offset,
                        ap=[[0, P]] + list(abv.ap[-1:])))

        # Persistent activation tensors
        kvq = top.enter_context(tc.tile_pool(name="kvq", bufs=1))
        kT = [_alloc(kvq, NGROUP, [P, TOWN], BF16, f"kT{h}_")
              for h in range(2)]
        vall = kvq.tile([P, TT_FULL, NGROUP, 130], BF16, name="vall")
        nc.gpsimd.memset(vall, 1.0)   # ones columns for softmax denominators
        qp1 = top.enter_context(tc.tile_pool(name="qp1", bufs=1))
        atp0 = top.enter_context(tc.tile_pool(name="atp0", bufs=1))
        atp1 = top.enter_context(tc.tile_pool(name="atp1", bufs=1))
        attnT = [_alloc(atp0, NGROUP, [P, 512], BF16, "attnT0_"),
                 _alloc(atp1, NGROUP, [P, 512], BF16, "attnT1_")]

        def ln_block(ctx_pools, x_ap_of, dst, dst_sl, g_col, b_col, pfx,
                     st_tags=("ssum", "ssq"), st_bufs=2):
            """LayerNorm one 512-token block (feature-major, bf16 inputs).

            x_ap_of(kt) -> [P,512] bf16 AP. dst: FT_C tiles, written at
            [:, dst_sl] in bf16. Stats via ones-matmul partition
            reductions; rstd = exp(-0.5*ln(var+eps))."""
            st_ps, rowp, bcp = ctx_pools
            xs = [x_ap_of(kt) for kt in range(KT_C)]
            ssum = st_ps.tile([1, 512], F32, tag=st_tags[0], name=f"{pfx}ss",
                              bufs=st_bufs)
            ssq = st_ps.tile([1, 512], F32, tag=st_tags[1], name=f"{pfx}sq",
                             bufs=st_bufs)
            for kt in range(KT_C):
                nc.tensor.matmul(ssum, ones1b, xs[kt],
                                 start=(kt == 0), stop=(kt == KT_C - 1))
            for kt in range(KT_C):
                sq = rowp.tile([P, 512], BF16, tag="sqt", name=f"{pfx}sqt{kt}",
                               bufs=2)
                nc.vector.tensor_mul(out=sq, in0=xs[kt], in1=xs[kt])
                nc.tensor.matmul(ssq, ones1b, sq,
                                 start=(kt == 0), stop=(kt == KT_C - 1))
            mu = rowp.tile([1, 512], F32, tag="mu", name=f"{pfx}mu", bufs=1)
            nc.vector.tensor_scalar_mul(out=mu, in0=ssum, scalar1=1.0 / C)
            var = rowp.tile([1, 512], F32, tag="var", name=f"{pfx}var",
                            bufs=1)
            nc.vector.tensor_mul(out=var, in0=mu, in1=mu)
            nc.vector.scalar_tensor_tensor(out=var, in0=ssq, scalar=1.0 / C,
                                           in1=var, op0=MULT, op1=SUB)
            nc.scalar.activation(out=var, in_=var, func=Ln,
                                 bias=eps_t[0:1, 0:1], scale=1.0)
            rs = rowp.tile([1, 512], BF16, tag="rs", name=f"{pfx}rs", bufs=1)
            nc.scalar.activation(out=rs, in_=var, func=Exp, scale=-0.5)
            ms = rowp.tile([1, 512], BF16, tag="ms", name=f"{pfx}ms", bufs=1)
            nc.vector.tensor_mul(out=ms, in0=mu, in1=rs)
            rs_b = bcp.tile([P, 512], BF16, tag="rsb", name=f"{pfx}rsb")
            nc.gpsimd.partition_broadcast(rs_b, rs)
            ms_b = bcp.tile([P, 512], BF16, tag="msb", name=f"{pfx}msb")
            nc.gpsimd.partition_broadcast(ms_b, ms)
            for ft in range(FT_C):
                t = rowp.tile([P, 512], BF16, tag="ap", name=f"{pfx}ap{ft}")
                nc.vector.tensor_mul(out=t, in0=xs[ft], in1=rs_b)
                nc.vector.tensor_sub(out=t, in0=t, in1=ms_b)
                nc.scalar.activation(out=dst[ft][:, dst_sl], in_=t,
                                     func=Ident, bias=b_col[:, ft:ft + 1],
                                     scale=g_col[:, ft:ft + 1])

        # ================= Phase A: LN1 (all 4 blocks) =================
        # x arrives as one [P, 8, 512] tile per 512-token block (single
        # DMA); squares and the (x*rs - ms) apply run as single wide DVE
        # ops over all 8 feature tiles, with the per-token rows broadcast
        # along the middle dim via stride-0 APs.
        xTr = xT.rearrange("(k p) t -> p k t", p=P)

        def rep8(ap):
            return bass.AP(tensor=ap.tensor, offset=ap.offset,
                           ap=[list(ap.ap[0]), [0, KT_C], list(ap.ap[-1])])

        sA = ExitStack()
        hfp = sA.enter_context(tc.tile_pool(name="hfp", bufs=1))
        hT = [_alloc(hfp, FT_C, [P, TOWN], BF16, f"hT{h}_") for h in range(2)]
        with ExitStack() as cl:
            st_ps = cl.enter_context(tc.tile_pool(name="st1", bufs=2,
                                                  space="PSUM"))
            rowp = cl.enter_context(tc.tile_pool(name="rowp1", bufs=3))
            bcp = cl.enter_context(tc.tile_pool(name="bcp1", bufs=2))
            lnp = cl.enter_context(tc.tile_pool(name="lnp", bufs=1))
            for half in range(2):
                for blk in range(2):
                    sl = slice(blk * 512, (blk + 1) * 512)
                    gb = half * 2 + blk
                    pfx = f"l1{half}{blk}"
                    x_all = lnp.tile([P, KT_C, 512], BF16, tag="xa",
                                     name=f"xa{gb}", bufs=2)
                    nc.sync.dma_start(
                        out=x_all,
                        in_=xTr[:, :, half * TOWN + blk * 512:
                                half * TOWN + (blk + 1) * 512])
                    ssum = st_ps.tile([1, 512], F32, tag="ssum",
                                      name=f"{pfx}ss")
                    ssq = st_ps.tile([1, 512], F32, tag="ssq",
                                     name=f"{pfx}sq")
                    for kt in range(KT_C):
                        nc.tensor.matmul(ssum, ones1b, x_all[:, kt, :],
                                         start=(kt == 0),
                                         stop=(kt == KT_C - 1))
                    sq_all = rowp.tile([P, KT_C, 512], BF16, tag="sqa",
                                       name=f"{pfx}sqa", bufs=1)
                    nc.vector.tensor_mul(out=sq_all, in0=x_all, in1=x_all)
                    for kt in range(KT_C):
                        nc.tensor.matmul(ssq, ones1b, sq_all[:, kt, :],
                                         start=(kt == 0),
                                         stop=(kt == KT_C - 1))
                    mu = rowp.tile([1, 512], F32, tag="mu", name=f"{pfx}mu",
                                   bufs=1)
                    nc.vector.tensor_scalar_mul(out=mu, in0=ssum,
                                                scalar1=1.0 / C)
                    var = rowp.tile([1, 512], F32, tag="var",
                                    name=f"{pfx}var", bufs=1)
                    nc.vector.tensor_mul(out=var, in0=mu, in1=mu)
                    nc.vector.scalar_tensor_tensor(
                        out=var, in0=ssq, scalar=1.0 / C, in1=var,
                        op0=MULT, op1=SUB)
                    nc.scalar.activation(out=var, in_=var, func=Ln,
                                         bias=eps_t[0:1, 0:1], scale=1.0)
                    rs = rowp.tile([1, 512], BF16, tag="rs",
                                   name=f"{pfx}rs", bufs=1)
                    nc.scalar.activation(out=rs, in_=var, func=Exp,
                                         scale=-0.5)
                    ms = rowp.tile([1, 512], BF16, tag="ms",
                                   name=f"{pfx}ms", bufs=1)
                    nc.vector.tensor_mul(out=ms, in0=mu, in1=rs)
                    rs_b = bcp.tile([P, 512], BF16, tag="rsb",
                                    name=f"{pfx}rsb")
                    nc.gpsimd.partition_broadcast(rs_b, rs)
                    ms_b = bcp.tile([P, 512], BF16, tag="msb",
                                    name=f"{pfx}msb")
                    nc.gpsimd.partition_broadcast(ms_b, ms)
                    t_all = rowp.tile([P, KT_C, 512], BF16, tag="ta",
                                      name=f"{pfx}ta", bufs=1)
                    nc.vector.tensor_mul(out=t_all, in0=x_all,
                                         in1=rep8(rs_b[:, :]))
                    nc.vector.tensor_sub(out=t_all, in0=t_all,
                                         in1=rep8(ms_b[:, :]))
                    for ft in range(FT_C):
                        nc.scalar.activation(
                            out=hT[half][ft][:, sl], in_=t_all[:, ft, :],
                            func=Ident, bias=ln1b_t[:, ft:ft + 1],
                            scale=ln1g_t[:, ft:ft + 1])

        # ============ Q + K/V projections (emission units) ============
        sW = ExitStack()
        ws = sW.enter_context(tc.tile_pool(name="ws", bufs=1))
        mm_kvq = sW.enter_context(tc.tile_pool(name="mm_kvq", bufs=2,
                                               space="PSUM"))
        # V weights: shared by all slots, resident
        wv = [[None, None] for _ in range(KT_C)]
        for kt in range(KT_C):
            for nb in range(2):
                w = ws.tile([P, 512], BF16, tag=f"wv{kt}_{nb}",
                            name=f"wv{kt}_{nb}", bufs=1)
                nc.sync.dma_start(
                    out=w, in_=attn_w[kt * P:(kt + 1) * P,
                                      2 * C + nb * 512:2 * C + (nb + 1) * 512])
                wv[kt][nb] = w

        sQ0 = ExitStack()
        qp0 = sQ0.enter_context(tc.tile_pool(name="qp0", bufs=1))
        qT = [_alloc(qp0, NGROUP, [P, 512], BF16, "qT0_"),
              _alloc(qp1, NGROUP, [P, 512], BF16, "qT1_")]

        def emit_q():
            wqs = {}

            def qload(g):
                wq = ws.tile([P, C], BF16, tag="wqg", name=f"wq{g}", bufs=2)
                nc.sync.dma_start(out=wq, in_=q_wp[g])
                wqs[g] = wq
            qload(0)
            for g in range(NGROUP):
                wq = wqs.pop(g)
                if g + 1 < NGROUP:
                    qload(g + 1)
                for qc in range(2):
                    ps = mm_kvq.tile([P, 512], F32, tag="mm",
                                     name=f"qps{g}_{qc}")
                    for kt in range(KT_C):
                        nc.tensor.matmul(
                            ps, wq[:, kt * P:(kt + 1) * P],
                            hT[0][kt][:, qc * 512:(qc + 1) * 512],
                            start=(kt == 0), stop=(kt == KT_C - 1))
                    nc.vector.tensor_scalar_add(
                        out=qT[qc][g], in0=ps, scalar1=abq_t[:, g:g + 1])

        def k_units(slots):
            """One unit per g: K columns for the given slots (wk loaded
            once, next group's load pre-issued)."""
            units = []
            wks = {}

            def kload(g, s0):
                wk = ws.tile([P, C], BF16, tag="wkg",
                             name=f"wk{g}_{s0}", bufs=2)
                nc.sync.dma_start(out=wk, in_=k_wp[g])
                wks[g] = wk
            for g in range(NGROUP):
                def go(_g=g, _slots=slots):
                    if _g not in wks:
                        kload(_g, _slots[0])
                    wk = wks.pop(_g)
                    if _g + 1 < NGROUP:
                        kload(_g + 1, _slots[0])
                    for s in _slots:
                        half, nb = s // 2, s % 2
                        ps = mm_kvq.tile([P, 512], F32, tag="mm",
                                         name=f"kps{_g}_{s}")
                        for kt in range(KT_C):
                            nc.tensor.matmul(
                                ps, wk[:, kt * P:(kt + 1) * P],
                                hT[half][kt][:, nb * 512:(nb + 1) * 512],
                                start=(kt == 0), stop=(kt == KT_C - 1))
                        nc.vector.tensor_scalar_add(
                            out=kT[half][_g][:, nb * 512:(nb + 1) * 512],
                            in0=ps, scalar1=abk_t[:, _g:_g + 1])
                units.append(go)
            return units

        def v_units(s):
            """One unit per (tt, nb): V rows for slot s."""
            units = []
            half, snb = s // 2, s % 2
            for tt in range(4):
                ht = snb * 4 + tt            # token tile within the half
                gt = half * FT_C + ht        # global token tile
                for nb in range(2):
                    def go(_ht=ht, _gt=gt, _nb=nb, _half=half):
                        ps = mm_kvq.tile([P, 512], F32, tag="mm",
                                         name=f"vps{_gt}_{_nb}")
                        for kt in range(KT_C):
                            nc.tensor.matmul(
                                ps, hT[_half][kt][:, _ht * P:(_ht + 1) * P],
                                wv[kt][_nb],
                                start=(kt == 0), stop=(kt == KT_C - 1))
                        dst = vall[:, _gt, 4 * _nb:4 * (_nb + 1), :]\
                            .rearrange("p g (h x) -> p (g h) x", h=2)
                        nc.vector.tensor_add(
                            out=dst[:, :, 0:64],
                            in0=ps.rearrange("p (h d) -> p h d", d=64),
                            in1=bv_bc[:, _nb * 512:(_nb + 1) * 512]
                            .rearrange("p (h d) -> p h d", d=64))
                    units.append(go)
            return units

        emit_q()
        for u in k_units([0, 2]):
            u()
        for u in v_units(0):
            u()
        for u in v_units(2):
            u()
        # K/V for slots 1 and 3: interleaved into attention qc0 below
        kv13 = k_units([1, 3]) + v_units(1) + v_units(3)

        # ================= attention machinery =================

        def attn_phase(qc, kts, masked, mtiles, sc_ps, y_pool, ppool,
                       npool, between=None):
            """Attention for all head-pair groups, one query chunk, as a
            single software pipeline over (g, k2) steps: scores/exp run one
            step ahead of AV across group boundaries, so the in-order PE
            stream never waits on the exp chain or the y-bank recycle.

            kts: global kt list (pairs). masked: {k2 -> mask tile idx}.
            between(g): emission hook after group g's last scores step."""
            nk2 = len(kts) // 2
            ys = {}
            pts = {}

            def emit_scores(g, k2):
                if k2 == 0:
                    ys[g] = [y_pool.tile([65, 512], F32, tag=f"y{hh}",
                                         name=f"y{qc}_{g}_{hh}")
                             for hh in range(2)]
                for hh in range(2):
                    hsl = slice(64 * hh, 64 * (hh + 1))
                    sc = sc_ps.tile([P, 1024], F32, tag="sc",
                                    name=f"sc{qc}_{g}_{k2}_{hh}")
                    for j in range(2):
                        kt = kts[2 * k2 + j]
                        nc.tensor.matmul(
                            sc[:, j * 512:(j + 1) * 512],
                            kT[kt // 8][g][hsl, (kt % 8) * P:(kt % 8 + 1) * P],
                            qT[qc][g][hsl, :], start=True, stop=True,
                            tile_position=(64 * hh, 0))
                    pt = ppool.tile([P, 1024], BF16, tag="pt",
                                    name=f"pt{qc}_{g}_{k2}_{hh}")
                    nc.scalar.activation(out=pt, in_=sc, func=Exp,
                                         scale=SCALE)
                    if k2 in masked:
                        # multiplicative {0,1} mask on the exp'd probs
                        # (bf16 SBUF mul; the ones-column denominator sees
                        # masked values, so softmax stays exact)
                        nc.vector.tensor_mul(out=pt, in0=pt,
                                             in1=mtiles[masked[k2]])
                    pts[(g, k2, hh)] = pt

            def emit_av(g, k2):
                for hh in range(2):
                    pt = pts.pop((g, k2, hh))
                    for j in range(2):
                        kt = kts[2 * k2 + j]
                        nc.tensor.matmul(
                            ys[g][hh], vall[:, kt, g, 65 * hh:65 * hh + 65],
                            pt[:, j * 512:(j + 1) * 512],
                            start=(k2 == 0 and j == 0),
                            stop=(k2 == nk2 - 1 and j == 1))
                if k2 == nk2 - 1:
                    y = ys.pop(g)
                    for hh in range(2):
                        # copy out of PSUM first (frees the bank for the
                        # next group's AV). The denominator row goes to a
                        # partition-0 f32 tile: the custom-DVE reciprocal
                        # mis-lowers at base_partition 64 (2.8e6 rel err
                        # on HW), and needs an SBUF f32 operand.
                        yr = npool.tile([1, 512], F32, tag="r",
                                        name=f"yr{qc}_{g}_{hh}", bufs=2)
                        nc.vector.tensor_copy(out=yr, in_=y[hh][64:65, :])
                        yc = npool.tile([64, 512], BF16, tag="yc",
                                        name=f"yc{qc}_{g}_{hh}", bufs=2)
                        nc.vector.tensor_copy(out=yc, in_=y[hh][0:64, :])
                        r = npool.tile([1, 512], F32, tag="r",
                                       name=f"r{qc}_{g}_{hh}", bufs=2)
                        nc.vector.reciprocal_approx_fast(out=r, in_=yr)
                        rb = npool.tile([64, 512], F32, tag="rb",
                                        name=f"rb{qc}_{g}_{hh}")
                        nc.gpsimd.partition_broadcast(rb, r[0:1, :])
                        nc.vector.tensor_mul(
                            out=attnT[qc][g][64 * hh:64 * (hh + 1), :],
                            in0=yc, in1=rb)

            steps = [(g, k2) for g in range(NGROUP) for k2 in range(nk2)]
            prev = None
            for g, k2 in steps:
                emit_scores(g, k2)
                if prev is not None:
                    emit_av(*prev)
                prev = (g, k2)
                if between is not None and k2 == nk2 - 1:
                    between(g)
            emit_av(*prev)

        # ============== MLP chain (emission units, lazy tiles) ==============
        x2T = [None, None]
        h2T = [None, None]
        h1T = [None] * ND

        def mlp_units(qc, pools):
            """proj + LN2 + fc1 + fc2 for one 512-token chunk as emission
            units. Weight/input DMAs for unit i+1 are issued before unit
            i's compute so the DMA queues run a unit ahead of the PE."""
            (xop, mlp_ps, w1p, w2p, h1p, x2p, h2p, op, pwp) = pools
            units = []
            ld = {}

            def proj_load(ft):
                xo = xop.tile([P, 512], BF16, tag="xo",
                              name=f"xo{qc}_{ft}", bufs=2)
                nc.sync.dma_start(
                    out=xo, in_=xT[ft * P:(ft + 1) * P,
                                   qc * 512:(qc + 1) * 512])
                pwt = pwp.tile([P, C], BF16, tag="pw",
                               name=f"pw{qc}_{ft}", bufs=2)
                nc.sync.dma_start(out=pwt, in_=proj_wp[ft])
                ld[("p", ft)] = (xo, pwt)

            def proj_unit(ft):
                def go():
                    if x2T[qc] is None:
                        x2T[qc] = [None] * FT_C
                    if ("p", ft) not in ld:
                        proj_load(ft)
                    xo, pwt = ld.pop(("p", ft))
                    if ft + 1 < FT_C:
                        proj_load(ft + 1)
                    ps = mlp_ps.tile([P, 512], F32, tag="mm",
                                     name=f"prj{qc}_{ft}")
                    for kt in range(KT_C):
                        nc.tensor.matmul(
                            ps, pwt[:, kt * P:(kt + 1) * P],
                            attnT[qc][kt], start=(kt == 0),
                            stop=(kt == KT_C - 1))
                    x2 = x2p.tile([P, 512], BF16, tag=f"x2_{ft}",
                                  name=f"x2_{qc}_{ft}")
                    x2T[qc][ft] = x2
                    nc.vector.scalar_tensor_tensor(
                        out=x2, in0=ps, scalar=projb_t[:, ft:ft + 1],
                        in1=xo, op0=ADD, op1=ADD)
                return go
            for ft in range(FT_C):
                units.append(proj_unit(ft))

            def ln2_unit():
                h2T[qc] = _alloc(h2p, FT_C, [P, 512], BF16, "h2_",
                                 namepfx=f"h2_{qc}_")
                with ExitStack() as c2:
                    rowp = c2.enter_context(
                        tc.tile_pool(name=f"rowl2{qc}", bufs=2))
                    bcp = c2.enter_context(
                        tc.tile_pool(name=f"bcl2{qc}", bufs=1))
                    ln_block((mlp_ps, rowp, bcp),
                             lambda kt: x2T[qc][kt][:, :], h2T[qc],
                             slice(0, 512), ln2g_t, ln2b_t, f"l2{qc}",
                             st_tags=("mm", "mm"), st_bufs=2)
            units.append(ln2_unit)

            def fc1_load(q16):
                w1 = w1p.tile([P, 2048], BF16, tag="w1",
                              name=f"w1{qc}_{q16}", bufs=2)
                nc.sync.dma_start(out=w1, in_=fc1_wp[q16])
                ld[("1", q16)] = w1

            def fc1_unit(q16):
                def go():
                    if ("1", q16) not in ld:
                        fc1_load(q16)
                    w1 = ld.pop(("1", q16))
                    if q16 + 1 < 16:
                        fc1_load(q16 + 1)
                    for dl in range(2):
                        d = q16 * 2 + dl
                        ps = mlp_ps.tile([P, 512], F32, tag="mm",
                                         name=f"f1{qc}_{d}")
                        for kt in range(KT_C):
                            nc.tensor.matmul(
                                ps,
                                w1[:, kt * 256 + dl * P:
                                   kt * 256 + (dl + 1) * P],
                                h2T[qc][kt], start=(kt == 0),
                                stop=(kt == KT_C - 1))
                        h1 = h1p.tile([P, 512], BF16, tag=f"h1_{d}",
                                      name=f"h1_{qc}_{d}")
                        h1T[d] = h1
                        if qc == 0:
                            # B1: ACT is saturated with attention exps;
                            # bias+relu as one DVE tensor_scalar instead
                            nc.vector.tensor_scalar(
                                out=h1, in0=ps,
                                scalar1=fc1b_t[:, d:d + 1], scalar2=0.0,
                                op0=ADD, op1=mybir.AluOpType.max)
                        else:
                            nc.scalar.activation(out=h1, in_=ps, func=Relu,
                                                 bias=fc1b_t[:, d:d + 1],
                                                 scale=1.0)
                return go
            for q16 in range(16):
                units.append(fc1_unit(q16))

            def fc2_load(ft, dh):
                w2 = w2p.tile([P, 2048], BF16, tag="w2",
                              name=f"w2{qc}_{ft}_{dh}", bufs=3)
                nc.sync.dma_start(
                    out=w2, in_=fc2_wp[ft][:, dh * 2048:(dh + 1) * 2048])
                ld[("2", ft, dh)] = w2

            def fc2_unit(ft):
                def go():
                    for dh in range(2):
                        if ("2", ft, dh) not in ld:
                            fc2_load(ft, dh)
                    if ft + 1 < FT_C:
                        fc2_load(ft + 1, 0)
                    ps = mlp_ps.tile([P, 512], F32, tag="mm",
                                     name=f"f2{qc}_{ft}")
                    for dh in range(2):
                        if ("2", ft, dh) not in ld:
                            fc2_load(ft, dh)
                        w2 = ld.pop(("2", ft, dh))
                        for dl in range(16):
                            d = dh * 16 + dl
                            nc.tensor.matmul(ps, w2[:, dl * P:(dl + 1) * P],
                                             h1T[d], start=(d == 0),
                                             stop=(d == ND - 1))
                    o = op.tile([P, 512], F32, tag="o", name=f"o{qc}_{ft}",
                                bufs=1)
                    nc.vector.scalar_tensor_tensor(
                        out=o, in0=ps, scalar=fc2b_t[:, ft:ft + 1],
                        in1=x2T[qc][ft], op0=ADD, op1=ADD)
                    nc.sync.dma_start(
                        out=out[ft * P:(ft + 1) * P,
                                qc * 512:(qc + 1) * 512], in_=o)
                return go
            for ft in range(FT_C):
                units.append(fc2_unit(ft))
            return units

        # shared pools for the MLP chains (entered at the B0 tail, after the
        # phase-A pools release, so the emission-ordered allocator can reuse
        # their space; tiles allocated lazily)
        sPS = ExitStack()
        sC = ExitStack()

        # ====== Phase B0: attention qc0 (+ K/V slots 1,3) + proj/LN2 qc0 ====
        if True:
            with ExitStack() as ca0:
                mp0 = ca0.enter_context(tc.tile_pool(name="mp0", bufs=1))
                m0 = _alloc(mp0, 4, [P, 1024], BF16, "m0_")
                for k2 in range(4):
                    nc.sync.dma_start(out=m0[k2],
                                      in_=mask0[k2 * P:(k2 + 1) * P, :])
                sc0 = ca0.enter_context(tc.tile_pool(name="sc0", bufs=2,
                                                     space="PSUM"))
                y0 = ca0.enter_context(tc.tile_pool(name="y0", bufs=1,
                                                    space="PSUM"))
                pp0 = ca0.enter_context(tc.tile_pool(name="pp0", bufs=4))
                np0 = ca0.enter_context(tc.tile_pool(name="np0", bufs=2))
                kst = {"i": 0}

                def kv_between(g):
                    nxt = (g + 1) * len(kv13) // NGROUP
                    while kst["i"] < nxt:
                        kv13[kst["i"]]()
                        kst["i"] += 1
                attn_phase(0, QC0_KT, {0: 0, 1: 1, 2: 2, 3: 3}, m0,
                           sc0, y0, pp0, np0, between=kv_between)
            sQ0.close()
            sW.close()   # free hT / wv / K-Q weight streams, mm_kvq PSUM
            sA.close()
            mlp_ps = sPS.enter_context(tc.tile_pool(name="mlp_ps", bufs=2,
                                                    space="PSUM"))
            xop = sC.enter_context(tc.tile_pool(name="xop", bufs=1))
            w1p = sC.enter_context(tc.tile_pool(name="w1p", bufs=1))
            w2p = sC.enter_context(tc.tile_pool(name="w2p", bufs=1))
            h1p = sC.enter_context(tc.tile_pool(name="h1p", bufs=1))
            x2p = sC.enter_context(tc.tile_pool(name="x2p", bufs=1))
            h2p = sC.enter_context(tc.tile_pool(name="h2p", bufs=1))
            op = sC.enter_context(tc.tile_pool(name="op", bufs=1))
            pwp = sC.enter_context(tc.tile_pool(name="pwp", bufs=1))
            pools = (xop, mlp_ps, w1p, w2p, h1p, x2p, h2p, op, pwp)
            units0 = mlp_units(0, pools)

        # == Phase B1: attention qc1 interleaved with the whole qc0 chain ==
        if True:
            with ExitStack() as ca1:
                mp1 = ca1.enter_context(tc.tile_pool(name="mp1", bufs=1))
                m1 = _alloc(mp1, 4, [P, 1024], BF16, "m1_")
                for i in range(4):
                    nc.sync.dma_start(out=m1[i],
                                      in_=mask1[i * P:(i + 1) * P, :])
                sc1 = ca1.enter_context(tc.tile_pool(name="sc1", bufs=2,
                                                     space="PSUM"))
                y1 = ca1.enter_context(tc.tile_pool(name="y1", bufs=1,
                                                    space="PSUM"))
                pp1 = ca1.enter_context(tc.tile_pool(name="pp1", bufs=3))
                np1 = ca1.enter_context(tc.tile_pool(name="np1", bufs=2))
                # last 4 fc2(qc0) units held back to fill the C prologue
                rest = units0[:-4]
                held = units0[-4:]
                ust = {"i": 0}

                def mlp_between(g):
                    nxt = (g + 1) * len(rest) // NGROUP
                    while ust["i"] < nxt:
                        rest[ust["i"]]()
                        ust["i"] += 1
                attn_phase(1, list(range(16)), QC1_MASKED_K2, m1,
                           sc1, y1, pp1, np1, between=mlp_between)
                while ust["i"] < len(rest):
                    rest[ust["i"]]()
                    ust["i"] += 1

            # ---- Phase C: proj + LN2 + MLP for chunk 1, with the held
            # back qc0 units filling the proj/LN2 dependency pocket ----
            units1 = mlp_units(1, pools)
            for u in held:      # ready immediately; cover C's serial start
                u()
            for u in units1:
                u()
        sC.close()
        sPS.close()

    nc.compile()
    return nc


_NC_CACHE = None


def _get_nc():
    global _NC_CACHE
    if _NC_CACHE is None:
        _NC_CACHE = build_nc()
    return _NC_CACHE


# core c = 2*b + j; j -> (cl, ch); perm = [cl, ch, rest ascending]
_CHUNKS = {0: (0, 3), 1: (1, 2)}
_PERMS = {0: (0, 3, 1, 2), 1: (1, 2, 0, 3)}


def _pair_mask(m):
    # [1024, 512] -> [512, 1024]: row-block k2 holds [mask(2*k2) | mask(2*k2+1)]
    return np.ascontiguousarray(
        m.reshape(4, 2, 128, 512).transpose(0, 2, 1, 3).reshape(512, 1024))


def _make_masks(perm):
    gpos = np.concatenate([np.arange(c * 512, (c + 1) * 512) for c in perm])
    q0 = gpos[0:512]
    q1 = gpos[512:1024]
    k0 = np.concatenate([gpos[0:512], gpos[1024:1536]])
    k1 = np.concatenate([gpos[512:1024], gpos[1536:2048]])
    m0 = np.where(k0[:, None] <= q0[None, :], 1.0, 0.0).astype(np.float32)
    m1 = np.where(k1[:, None] <= q1[None, :], 1.0, 0.0).astype(np.float32)
    return _pair_mask(m0), _pair_mask(m1)


def _run(inputs, trace=False):
    nc = _get_nc()
    xs = {k: np.ascontiguousarray(np.asarray(v), dtype=np.float32)
          for k, v in inputs.items()}
    bf = lambda a: np.ascontiguousarray(a.astype(ml_dtypes.bfloat16))
    aw = xs["attn_w"]
    # [g, p, kt*128+f] = attn_w[kt*128+p, sec + g*128+f]
    q_wp = bf(aw[:, 0:C].reshape(KT_C, P, NGROUP, P)
              .transpose(2, 1, 0, 3).reshape(NGROUP, P, C))
    k_wp = bf(aw[:, C:2 * C].reshape(KT_C, P, NGROUP, P)
              .transpose(2, 1, 0, 3).reshape(NGROUP, P, C))
    # [ft, p, kt*128+f] = proj_w[kt*128+p, ft*128+f]
    proj_wp = bf(xs["proj_w"].reshape(KT_C, P, FT_C, P)
                 .transpose(2, 1, 0, 3).reshape(FT_C, P, C))
    # [q, p, kt*256+f] = fc1_w[kt*128+p, q*256+f]
    fc1_wp = bf(xs["fc1_w"].reshape(KT_C, P, 16, 256)
                .transpose(2, 1, 0, 3).reshape(16, P, 2048))
    # [ft, p, d*128+f] = fc2_w[d*128+p, ft*128+f]
    fc2_wp = bf(xs["fc2_w"].reshape(ND, P, FT_C, P)
                .transpose(2, 1, 0, 3).reshape(FT_C, P, DFF))
    attn_w_bf = bf(aw)
    x = xs["x"]
    in_maps = []
    for c in range(8):
        b, j = divmod(c, 2)
        perm = _PERMS[j]
        m0, m1 = _make_masks(perm)
        xTp = bf(np.concatenate(
            [x[b][p * 512:(p + 1) * 512] for p in perm], axis=0).T)
        in_maps.append({
            "xT": xTp, "mask0": bf(m0), "mask1": bf(m1),
            "attn_w": attn_w_bf, "q_wp": q_wp, "k_wp": k_wp,
            "attn_b": xs["attn_b"],
            "proj_wp": proj_wp, "proj_b": xs["proj_b"],
            "ln1_g": xs["ln1_g"], "ln1_b": xs["ln1_b"],
            "ln2_g": xs["ln2_g"], "ln2_b": xs["ln2_b"],
            "fc1_wp": fc1_wp, "fc1_b": xs["fc1_b"],
            "fc2_wp": fc2_wp, "fc2_b": xs["fc2_b"],
        })
    res = run_bass_kernel_spmd(nc, in_maps, list(range(8)), trace=trace)
    full = np.empty((B, T, C), dtype=np.float32)
    for c in range(8):
        b, j = divmod(c, 2)
        cl, ch = _CHUNKS[j]
        o = res.results[c]["out"]            # [C, TOWN] feature-major
        full[b, cl * 512:(cl + 1) * 512] = o[:, 0:512].T
        full[b, ch * 512:(ch + 1) * 512] = o[:, 512:1024].T
    return full, res.exec_time_ns


def kernel(**inputs):
    out, _ = _run(inputs, trace=False)
    return out
